# revision 18
# baseline (speedup 1.0000x reference)
"""Windowed (Swin-style) multi-head attention on 8 TRN2 NeuronCores.

Data-parallel: 256 independent windows -> 32 per core. Per window:
  qkv = x @ w_qkv ; per-head attn = softmax(q k^T * scale + bias) ; out = (attn v) @ w_proj + b_proj

Device-side layout strategy (all matmuls contract over the partition dim):
  - host pre-transposes x to channel-major xT[c, tok] so qT/kT are produced
    feature-major (ready to be score-matmul operands) and v token-major.
  - scores are computed TRANSPOSED, S^T[k, q] (lhsT = kT tile, rhs = qT), so
    softmax normalization runs over the partition axis:
      exp via ScalarE (scale folded in), * exp(bias) via VectorE,
      column-sums via ones-vector matmul on TensorE,
      reciprocal on VectorE, broadcast back to feature rows via a tiny
      indicator matmul (E[h,f] = 1 iff feature f belongs to head h).
  - avT[f, q] = v-contracted matmul accumulated over k tiles; normalized
    avT is exactly the lhsT the projection matmul needs. b_proj is seeded
    into PSUM with a rank-1 ones x b_proj matmul.
Matmul operands are bf16 (full-rate PE, fp32 PSUM accumulation); inputs are
rounded to bf16 on the host so they can be DMA'd directly.
"""

import sys

for _p in ("/opt/trn_rl_repo",):
    if _p not in sys.path:
        sys.path.insert(0, _p)

import ml_dtypes
import numpy as np
from contextlib import ExitStack

import concourse.bass as bass
import concourse.bacc as bacc
import concourse.mybir as mybir
from concourse import tile
from concourse.bass_utils import run_bass_kernel_spmd

NCORES = 8
BS = 256
W = BS // NCORES  # windows per core
N = 256           # tokens per window
DIM = 512
NH = 8
HD = 64
SCALE = HD ** -0.5
F32 = mybir.dt.float32
BF16 = mybir.dt.bfloat16
NPBF = ml_dtypes.bfloat16
EXP = mybir.ActivationFunctionType.Exp
COPY = mybir.ActivationFunctionType.Copy


def build(w_count=W):
    nc = bacc.Bacc(None, target_bir_lowering=False)
    xt = nc.declare_dram_parameter("xt", [w_count, DIM, N], BF16, False)
    wqk = nc.declare_dram_parameter("wqk", [DIM, 2 * DIM], BF16, False)
    wv = nc.declare_dram_parameter("wv", [DIM, DIM], BF16, False)
    wp = nc.declare_dram_parameter("wp", [DIM, DIM], BF16, False)
    bp = nc.declare_dram_parameter("bp", [1, DIM], BF16, False)
    ebt = nc.declare_dram_parameter("ebt", [NH, 2, 128, N], BF16, False)
    ein = nc.declare_dram_parameter("ein", [128, 4, 128], BF16, False)
    ones_c = nc.declare_dram_parameter("ones_c", [128, 32], BF16, False)
    ones_r = nc.declare_dram_parameter("ones_r", [1, 128], BF16, False)
    out = nc.declare_dram_parameter("out", [w_count, N, DIM], F32, True)

    with ExitStack() as ctx:
        tc = ctx.enter_context(tile.TileContext(nc))
        const = ctx.enter_context(tc.tile_pool(name="const", bufs=1))
        p_xt = ctx.enter_context(tc.tile_pool(name="xt", bufs=2))
        p_qk = ctx.enter_context(tc.tile_pool(name="qk", bufs=2))
        p_v = ctx.enter_context(tc.tile_pool(name="v", bufs=2))
        p_p = ctx.enter_context(tc.tile_pool(name="pp", bufs=2))
        p_e = ctx.enter_context(tc.tile_pool(name="te", bufs=4))
        p_bc = ctx.enter_context(tc.tile_pool(name="bc", bufs=2))
        p_av = ctx.enter_context(tc.tile_pool(name="av", bufs=2))
        p_rs = ctx.enter_context(tc.tile_pool(name="rs", bufs=2))
        p_o = ctx.enter_context(tc.tile_pool(name="os", bufs=3))
        ps = ctx.enter_context(tc.tile_pool(name="ps", bufs=4, space="PSUM"))
        ps2 = ctx.enter_context(tc.tile_pool(name="ps2", bufs=1, space="PSUM"))

        wqk_s = const.tile([128, 4, 2 * DIM], BF16)
        nc.sync.dma_start(wqk_s[:], wqk.ap().rearrange("(t p) f -> p t f", p=128))
        wv_s = const.tile([128, 4, DIM], BF16)
        nc.sync.dma_start(wv_s[:], wv.ap().rearrange("(t p) f -> p t f", p=128))
        wp_s = const.tile([128, 4, DIM], BF16)
        nc.sync.dma_start(wp_s[:], wp.ap().rearrange("(t p) f -> p t f", p=128))
        bp_s = const.tile([1, DIM], BF16)
        nc.sync.dma_start(bp_s[:], bp.ap())
        eb_s = const.tile([128, 2 * NH, N], BF16)
        nc.sync.dma_start(eb_s[:], ebt.ap().rearrange("h t p q -> p (h t) q"))
        ei_s = const.tile([128, 4, 128], BF16)
        nc.sync.dma_start(ei_s[:], ein.ap())
        oc_s = const.tile([128, 32], BF16)
        nc.sync.dma_start(oc_s[:], ones_c.ap())
        or_s = const.tile([1, 128], BF16)
        nc.sync.dma_start(or_s[:], ones_r.ap())

        xt_ap = xt.ap()
        out_ap = out.ap()

        for w in range(w_count):
            # load xT (channel-major) as [128, ctile, tok]
            xt_s = p_xt.tile([128, 4, N], BF16, tag="xt")
            nc.sync.dma_start(
                xt_s[:], xt_ap[w].rearrange("(t p) q -> p t q", p=128)
            )

            # qkT[feat, tok] for the q,k feature block (first 1024 features)
            qk_s = p_qk.tile([128, 8, N], BF16, tag="qk")
            for ft in range(8):
                acc = ps.tile([128, 512], F32, tag="ps")
                for ct in range(4):
                    nc.tensor.matmul(
                        acc[:, 0:N],
                        wqk_s[:, ct, ft * 128:(ft + 1) * 128],
                        xt_s[:, ct, :],
                        start=(ct == 0),
                        stop=(ct == 3),
                    )
                nc.vector.tensor_copy(qk_s[:, ft, :], acc[:, 0:N])

            # v[tok, feat] (token-major)
            v_s = p_v.tile([128, 2, DIM], BF16, tag="v")
            for kt in range(2):
                acc = ps.tile([128, 512], F32, tag="ps")
                for ct in range(4):
                    nc.tensor.matmul(
                        acc[:],
                        xt_s[:, ct, kt * 128:(kt + 1) * 128],
                        wv_s[:, ct, :],
                        start=(ct == 0),
                        stop=(ct == 3),
                    )
                nc.vector.tensor_copy(v_s[:, kt, :], acc[:])

            # scores^T per head -> exp(scale*s) * exp(bias); rowsums via ones-matmul.
            # Rowsums land as 32 replicated rows at partition 32*(h%4):
            # heads 0-3 fill rsA, heads 4-7 fill rsB.
            pp_s = p_p.tile([128, 2 * NH, N], BF16, tag="pp")
            rsA = ps2.tile([128, N], F32, tag="rsA")
            rsB = ps2.tile([128, N], F32, tag="rsB")
            for h in range(NH):
                qft = h // 2
                base = (h % 2) * HD
                sc = ps.tile([128, 512], F32, tag="ps")
                for kt in range(2):
                    nc.tensor.matmul(
                        sc[:, kt * N:(kt + 1) * N],
                        qk_s[base:base + HD, 4 + qft, kt * 128:(kt + 1) * 128],
                        qk_s[base:base + HD, qft, :],
                        start=True,
                        stop=True,
                    )
                for kt in range(2):
                    te = p_e.tile([128, N], BF16, tag="te")
                    nc.scalar.activation(
                        te[:], sc[:, kt * N:(kt + 1) * N], EXP, scale=SCALE
                    )
                    nc.vector.tensor_mul(
                        pp_s[:, 2 * h + kt, :], te[:], eb_s[:, 2 * h + kt, :]
                    )
                rr = rsA if h < 4 else rsB
                pos = 32 * (h % 4)
                for kt in range(2):
                    nc.tensor.matmul(
                        rr[pos:pos + 32, :],
                        oc_s[:],
                        pp_s[:, 2 * h + kt, :],
                        start=(kt == 0),
                        stop=(kt == 1),
                        tile_position=(0, pos),
                    )

            # reciprocal of denominators; broadcast to feature rows via E-matmul
            # (E[p, f] = 1 iff p == 32 * (head(f) % 4), rhs = the recip tile)
            rcA = p_rs.tile([128, N], BF16, tag="rcA")
            rcB = p_rs.tile([128, N], BF16, tag="rcB")
            with nc.allow_low_precision(reason="bf16 softmax denom ok at 2e-2 gate"):
                nc.vector.reciprocal(rcA[:], rsA[:])
                nc.vector.reciprocal(rcB[:], rsB[:])
            bc_s = p_bc.tile([128, 4, N], F32, tag="bc")
            for ftl in range(4):
                bb = ps.tile([128, 512], F32, tag="ps")
                nc.tensor.matmul(
                    bb[:, 0:N],
                    ei_s[:, ftl, :],
                    (rcA if ftl < 2 else rcB)[:],
                    start=True,
                    stop=True,
                )
                nc.scalar.activation(bc_s[:, ftl, :], bb[:, 0:N], COPY)

            # avT[f, q]: pairs of heads packed into PE column groups; then normalize
            av_s = p_av.tile([128, 4, N], BF16, tag="av")
            for ftl in range(4):
                aa = ps.tile([128, 512], F32, tag="ps")
                for sub in range(2):
                    h = 2 * ftl + sub
                    for kt in range(2):
                        nc.tensor.matmul(
                            aa[sub * HD:(sub + 1) * HD, 0:N],
                            v_s[:, kt, h * HD:(h + 1) * HD],
                            pp_s[:, 2 * h + kt, :],
                            start=(kt == 0),
                            stop=(kt == 1),
                            tile_position=(0, sub * HD),
                        )
                nc.vector.tensor_mul(av_s[:, ftl, :], aa[:, 0:N], bc_s[:, ftl, :])

            # projection (+ rank-1 b_proj seed); copy to SBUF, DMA out
            for qt in range(2):
                oo = ps.tile([128, 512], F32, tag="ps")
                nc.tensor.matmul(
                    oo[:], or_s[:], bp_s[:], start=True, stop=False
                )
                for ftl in range(4):
                    nc.tensor.matmul(
                        oo[:],
                        av_s[:, ftl, qt * 128:(qt + 1) * 128],
                        wp_s[:, ftl, :],
                        start=False,
                        stop=(ftl == 3),
                    )
                o_s = p_o.tile([128, DIM], F32, tag="os")
                nc.vector.tensor_copy(o_s[:], oo[:])
                nc.sync.dma_start(out_ap[w, qt * 128:(qt + 1) * 128, :], o_s[:])

    nc.finalize()
    return nc


_NC_CACHE = {}


def _get_nc(w_count):
    if w_count not in _NC_CACHE:
        _NC_CACHE[w_count] = build(w_count)
    return _NC_CACHE[w_count]


def _prep(inputs, w_count):
    x = np.asarray(inputs["x"], dtype=np.float32)
    noise = np.asarray(inputs["noise"], dtype=np.float32)
    ns = np.asarray(inputs["noise_strength"], dtype=np.float32)
    wqkv = np.asarray(inputs["w_qkv"], dtype=np.float32)
    wproj = np.asarray(inputs["w_proj"], dtype=np.float32)
    bproj = np.asarray(inputs["b_proj"], dtype=np.float32)
    bt = np.asarray(inputs["bias_table"], dtype=np.float32)
    ri = np.asarray(inputs["rel_index"])

    xe = x + noise * ns                                     # [BS, N, DIM]
    xt = np.ascontiguousarray(xe.transpose(0, 2, 1).astype(NPBF))
    eb = np.exp(bt[ri])                                     # [q, k, h]
    ebt = np.ascontiguousarray(
        eb.transpose(2, 1, 0).astype(NPBF)
    ).reshape(NH, 2, 128, N)
    ein = np.zeros((128, 4, 128), NPBF)
    for ftl in range(4):
        for fl in range(128):
            head = 2 * ftl + (fl // HD)
            ein[32 * (head % 4), ftl, fl] = 1.0

    common = {
        "wqk": np.ascontiguousarray(wqkv[:, : 2 * DIM].astype(NPBF)),
        "wv": np.ascontiguousarray(wqkv[:, 2 * DIM:].astype(NPBF)),
        "wp": np.ascontiguousarray(wproj.astype(NPBF)),
        "bp": np.ascontiguousarray(bproj.reshape(1, DIM).astype(NPBF)),
        "ebt": ebt,
        "ein": ein,
        "ones_c": np.ones((128, 32), NPBF),
        "ones_r": np.ones((1, 128), NPBF),
    }
    in_maps = []
    for i in range(NCORES):
        m = dict(common)
        m["xt"] = np.ascontiguousarray(xt[i * w_count:(i + 1) * w_count])
        in_maps.append(m)
    return in_maps


def _run(inputs, w_count=W, trace=False):
    nc = _get_nc(w_count)
    in_maps = _prep(inputs, w_count)
    res = run_bass_kernel_spmd(
        nc, in_maps, core_ids=list(range(NCORES)), trace=trace
    )
    full = np.concatenate([res.results[i]["out"] for i in range(NCORES)], axis=0)
    return full, res


def kernel(**inputs):
    out, _ = _run(inputs, W, trace=False)
    return out


def kernel_profiled(inputs, w_count=W):
    out, res = _run(inputs, w_count, trace=True)
    return out, res


# revision 22
# speedup vs baseline: 1.1511x; 1.1511x over previous
"""Windowed (Swin-style) multi-head attention on 8 TRN2 NeuronCores.

Data-parallel: 256 independent windows -> 32 per core. Per window:
  qkv = x @ w_qkv ; per-head attn = softmax(q k^T * scale + bias) ; out = (attn v) @ w_proj + b_proj

Device-side layout strategy (all matmuls contract over the partition dim):
  - host pre-transposes x to channel-major xT[c, tok] so qT/kT are produced
    feature-major (ready to be score-matmul operands) and v token-major.
  - scores are computed TRANSPOSED, S^T[k, q] (lhsT = kT tile, rhs = qT), so
    softmax normalization runs over the partition axis:
      exp via ScalarE (scale folded in), * exp(bias) via VectorE,
      column-sums via ones-block matmul on TensorE (32 replicated rows at
      32-aligned partitions), reciprocal_approx_fast on VectorE, broadcast
      back to feature rows via an indicator matmul.
  - avT[f, q] = v-contracted matmul accumulated over k tiles; normalized
    avT is exactly the lhsT the projection matmul needs. b_proj is added
    (pre-broadcast on host) during the PSUM->SBUF output copy.
Matmul operands are bf16 (full-rate PE, fp32 PSUM accumulation); inputs are
rounded to bf16 on the host so they can be DMA'd directly.
"""

import sys

for _p in ("/opt/trn_rl_repo",):
    if _p not in sys.path:
        sys.path.insert(0, _p)

import ml_dtypes
import numpy as np
from contextlib import ExitStack

import concourse.bass as bass
import concourse.bacc as bacc
import concourse.mybir as mybir
from concourse import tile
from concourse.bass_utils import run_bass_kernel_spmd

NCORES = 8
BS = 256
W = BS // NCORES  # windows per core
N = 256           # tokens per window
DIM = 512
NH = 8
HD = 64
SCALE = HD ** -0.5
F32 = mybir.dt.float32
BF16 = mybir.dt.bfloat16
NPBF = ml_dtypes.bfloat16
EXP = mybir.ActivationFunctionType.Exp
COPY = mybir.ActivationFunctionType.Copy


def build(w_count=W):
    nc = bacc.Bacc(None, target_bir_lowering=False)
    xt = nc.declare_dram_parameter("xt", [w_count, DIM, N], BF16, False)
    wqk = nc.declare_dram_parameter("wqk", [DIM, 2 * DIM], BF16, False)
    wv = nc.declare_dram_parameter("wv", [DIM, DIM], BF16, False)
    wp = nc.declare_dram_parameter("wp", [DIM, DIM], BF16, False)
    brep = nc.declare_dram_parameter("brep", [128, DIM], F32, False)
    ebt = nc.declare_dram_parameter("ebt", [NH, 2, 128, N], BF16, False)
    ein = nc.declare_dram_parameter("ein", [128, 4, 128], BF16, False)
    ones_c = nc.declare_dram_parameter("ones_c", [128, 32], BF16, False)
    out = nc.declare_dram_parameter("out", [w_count, N, DIM], F32, True)

    with ExitStack() as ctx:
        tc = ctx.enter_context(tile.TileContext(nc))
        const = ctx.enter_context(tc.tile_pool(name="const", bufs=1))
        p_xt = ctx.enter_context(tc.tile_pool(name="xt", bufs=3))
        p_qk = ctx.enter_context(tc.tile_pool(name="qk", bufs=3))
        p_v = ctx.enter_context(tc.tile_pool(name="v", bufs=3))
        p_p = ctx.enter_context(tc.tile_pool(name="pp", bufs=2))
        p_e = ctx.enter_context(tc.tile_pool(name="te", bufs=4))
        p_bc = ctx.enter_context(tc.tile_pool(name="bc", bufs=3))
        p_av = ctx.enter_context(tc.tile_pool(name="av", bufs=3))
        p_rs = ctx.enter_context(tc.tile_pool(name="rs", bufs=2))
        p_o = ctx.enter_context(tc.tile_pool(name="os", bufs=4))
        ps = ctx.enter_context(tc.tile_pool(name="ps", bufs=6, space="PSUM"))
        ps2 = ctx.enter_context(tc.tile_pool(name="ps2", bufs=1, space="PSUM"))

        wqk_s = const.tile([128, 4, 2 * DIM], BF16)
        nc.sync.dma_start(wqk_s[:], wqk.ap().rearrange("(t p) f -> p t f", p=128))
        wv_s = const.tile([128, 4, DIM], BF16)
        nc.sync.dma_start(wv_s[:], wv.ap().rearrange("(t p) f -> p t f", p=128))
        wp_s = const.tile([128, 4, DIM], BF16)
        nc.sync.dma_start(wp_s[:], wp.ap().rearrange("(t p) f -> p t f", p=128))
        br_s = const.tile([128, DIM], F32)
        nc.sync.dma_start(br_s[:], brep.ap())
        eb_s = const.tile([128, NH, 2, N], BF16)
        nc.sync.dma_start(eb_s[:], ebt.ap().rearrange("h t p q -> p h t q"))
        ei_s = const.tile([128, 4, 128], BF16)
        nc.sync.dma_start(ei_s[:], ein.ap())
        oc_s = const.tile([128, 32], BF16)
        nc.sync.dma_start(oc_s[:], ones_c.ap())

        xt_ap = xt.ap()
        out_ap = out.ap()

        for w in range(w_count):
            # load xT (channel-major) as [128, ctile, tok]
            xt_s = p_xt.tile([128, 4, N], BF16, tag="xt")
            nc.sync.dma_start(
                xt_s[:], xt_ap[w].rearrange("(t p) q -> p t q", p=128)
            )

            # qkT[feat, tok] for the q,k feature block (first 1024 features);
            # two feature tiles share one PSUM bank -> one batched cast on ACT
            qk_s = p_qk.tile([128, 8 * N], BF16, tag="qk")
            for fp in range(4):
                acc = ps.tile([128, 512], F32, tag="ps")
                for sub in range(2):
                    ft = 2 * fp + sub
                    for ct in range(4):
                        nc.tensor.matmul(
                            acc[:, sub * N:(sub + 1) * N],
                            wqk_s[:, ct, ft * 128:(ft + 1) * 128],
                            xt_s[:, ct, :],
                            start=(ct == 0),
                            stop=(ct == 3),
                        )
                nc.scalar.activation(
                    qk_s[:, fp * 512:(fp + 1) * 512], acc[:], COPY
                )

            # v[tok, feat] (token-major)
            v_s = p_v.tile([128, 2 * DIM], BF16, tag="v")
            for kt in range(2):
                acc = ps.tile([128, 512], F32, tag="ps")
                for ct in range(4):
                    nc.tensor.matmul(
                        acc[:],
                        xt_s[:, ct, kt * 128:(kt + 1) * 128],
                        wv_s[:, ct, :],
                        start=(ct == 0),
                        stop=(ct == 3),
                    )
                nc.vector.tensor_copy(v_s[:, kt * DIM:(kt + 1) * DIM], acc[:])

            # scores^T per head -> exp(scale*s) * exp(bias); rowsums via
            # ones-block matmul (32 replicated rows at partition 32*(h%4):
            # heads 0-3 fill rsA, heads 4-7 fill rsB).
            pp_s = p_p.tile([128, 2 * NH, N], BF16, tag="pp")
            rsA = ps2.tile([128, N], F32, tag="rsA")
            rsB = ps2.tile([128, N], F32, tag="rsB")
            for h in range(NH):
                qco = (h // 2) * N
                kco = (4 + h // 2) * N
                base = (h % 2) * HD
                sc = ps.tile([128, 2, N], F32, tag="ps")
                for kt in range(2):
                    nc.tensor.matmul(
                        sc[:, kt, :],
                        qk_s[base:base + HD, kco + kt * 128:kco + (kt + 1) * 128],
                        qk_s[base:base + HD, qco:qco + N],
                        start=True,
                        stop=True,
                    )
                te = p_e.tile([128, 2, N], BF16, tag="te")
                nc.scalar.activation(te[:], sc[:], EXP, scale=SCALE)
                nc.vector.tensor_mul(
                    pp_s[:, 2 * h:2 * h + 2, :], te[:], eb_s[:, h, :, :]
                )
                rr = rsA if h < 4 else rsB
                pos = 32 * (h % 4)
                for kt in range(2):
                    nc.tensor.matmul(
                        rr[pos:pos + 32, :],
                        oc_s[:],
                        pp_s[:, 2 * h + kt, :],
                        start=(kt == 0),
                        stop=(kt == 1),
                        tile_position=(0, pos),
                    )

            # reciprocal of denominators (fp32 fast-approx), one cast to bf16;
            # broadcast to feature rows via indicator matmul
            rc = p_rs.tile([128, 512], F32, tag="rc")
            nc.vector.reciprocal_approx_fast(rc[:, 0:N], rsA[:])
            nc.vector.reciprocal_approx_fast(rc[:, N:2 * N], rsB[:])
            rcb = p_rs.tile([128, 512], BF16, tag="rcb")
            nc.vector.tensor_copy(rcb[:], rc[:])
            bc_s = p_bc.tile([128, 4 * N], F32, tag="bc")
            for bp2 in range(2):
                bb = ps.tile([128, 512], F32, tag="ps")
                for sub in range(2):
                    ftl = 2 * bp2 + sub
                    nc.tensor.matmul(
                        bb[:, sub * N:(sub + 1) * N],
                        ei_s[:, ftl, :],
                        rcb[:, (ftl // 2) * N:(ftl // 2 + 1) * N],
                        start=True,
                        stop=True,
                    )
                nc.scalar.activation(
                    bc_s[:, bp2 * 512:(bp2 + 1) * 512], bb[:], COPY
                )

            # avT[f, q]: pairs of heads packed into PE column groups; two
            # feature tiles share a PSUM bank -> one batched normalize-mult
            av_s = p_av.tile([128, 4 * N], BF16, tag="av")
            for ap2 in range(2):
                aa = ps.tile([128, 512], F32, tag="ps")
                for sub2 in range(2):
                    ftl = 2 * ap2 + sub2
                    for sub in range(2):
                        h = 2 * ftl + sub
                        for kt in range(2):
                            nc.tensor.matmul(
                                aa[sub * HD:(sub + 1) * HD,
                                   sub2 * N:(sub2 + 1) * N],
                                v_s[:, kt * DIM + h * HD:kt * DIM + (h + 1) * HD],
                                pp_s[:, 2 * h + kt, :],
                                start=(kt == 0),
                                stop=(kt == 1),
                                tile_position=(0, sub * HD),
                            )
                nc.vector.tensor_mul(
                    av_s[:, ap2 * 512:(ap2 + 1) * 512],
                    aa[:],
                    bc_s[:, ap2 * 512:(ap2 + 1) * 512],
                )

            # projection; add (pre-broadcast) b_proj during PSUM->SBUF copy
            for qt in range(2):
                oo = ps.tile([128, 512], F32, tag="ps")
                for ftl in range(4):
                    nc.tensor.matmul(
                        oo[:],
                        av_s[:, ftl * N + qt * 128:ftl * N + qt * 128 + 128],
                        wp_s[:, ftl, :],
                        start=(ftl == 0),
                        stop=(ftl == 3),
                    )
                o_s = p_o.tile([128, DIM], F32, tag="os")
                nc.vector.tensor_add(o_s[:], oo[:], br_s[:])
                nc.sync.dma_start(out_ap[w, qt * 128:(qt + 1) * 128, :], o_s[:])

    nc.finalize()
    return nc


_NC_CACHE = {}


def _get_nc(w_count):
    if w_count not in _NC_CACHE:
        _NC_CACHE[w_count] = build(w_count)
    return _NC_CACHE[w_count]


def _prep(inputs, w_count):
    x = np.asarray(inputs["x"], dtype=np.float32)
    noise = np.asarray(inputs["noise"], dtype=np.float32)
    ns = np.asarray(inputs["noise_strength"], dtype=np.float32)
    wqkv = np.asarray(inputs["w_qkv"], dtype=np.float32)
    wproj = np.asarray(inputs["w_proj"], dtype=np.float32)
    bproj = np.asarray(inputs["b_proj"], dtype=np.float32)
    bt = np.asarray(inputs["bias_table"], dtype=np.float32)
    ri = np.asarray(inputs["rel_index"])

    xe = x + noise * ns                                     # [BS, N, DIM]
    xt = np.ascontiguousarray(xe.transpose(0, 2, 1).astype(NPBF))
    eb = np.exp(bt[ri])                                     # [q, k, h]
    ebt = np.ascontiguousarray(
        eb.transpose(2, 1, 0).astype(NPBF)
    ).reshape(NH, 2, 128, N)
    ein = np.zeros((128, 4, 128), NPBF)
    for ftl in range(4):
        for fl in range(128):
            head = 2 * ftl + (fl // HD)
            ein[32 * (head % 4), ftl, fl] = 1.0

    common = {
        "wqk": np.ascontiguousarray(wqkv[:, : 2 * DIM].astype(NPBF)),
        "wv": np.ascontiguousarray(wqkv[:, 2 * DIM:].astype(NPBF)),
        "wp": np.ascontiguousarray(wproj.astype(NPBF)),
        "brep": np.ascontiguousarray(
            np.broadcast_to(bproj.reshape(1, DIM), (128, DIM)).astype(np.float32)
        ),
        "ebt": ebt,
        "ein": ein,
        "ones_c": np.ones((128, 32), NPBF),
    }
    in_maps = []
    for i in range(NCORES):
        m = dict(common)
        m["xt"] = np.ascontiguousarray(xt[i * w_count:(i + 1) * w_count])
        in_maps.append(m)
    return in_maps


def _run(inputs, w_count=W, trace=False):
    nc = _get_nc(w_count)
    in_maps = _prep(inputs, w_count)
    res = run_bass_kernel_spmd(
        nc, in_maps, core_ids=list(range(NCORES)), trace=trace
    )
    full = np.concatenate([res.results[i]["out"] for i in range(NCORES)], axis=0)
    return full, res


def kernel(**inputs):
    out, _ = _run(inputs, W, trace=False)
    return out


def kernel_profiled(inputs, w_count=W):
    out, res = _run(inputs, w_count, trace=True)
    return out, res


# revision 36
# speedup vs baseline: 10.7792x; 9.3640x over previous
"""Windowed (Swin-style) multi-head attention on 8 TRN2 NeuronCores.

Data-parallel: 256 independent windows -> 32 per core. Per window:
  qkv = x @ w_qkv ; per-head attn = softmax(q k^T * scale + bias) ; out = (attn v) @ w_proj + b_proj

Device-side layout strategy (all matmuls contract over the partition dim):
  - host pre-transposes x to channel-major xT[c, tok] so qT/kT are produced
    feature-major (ready to be score-matmul operands) and v token-major.
  - scores are computed TRANSPOSED, S^T[k, q] (lhsT = kT tile, rhs = qT), so
    softmax normalization runs over the partition axis:
      exp via ScalarE (scale folded in), * exp(bias) via VectorE,
      column-sums via ones-block matmul on TensorE (32 replicated rows at
      32-aligned partitions), reciprocal_approx_fast on VectorE, broadcast
      back to feature rows via an indicator matmul.
  - avT[f, q] = v-contracted matmul accumulated over k tiles; normalized
    avT is exactly the lhsT the projection matmul needs. b_proj is added
    (pre-broadcast on host) during the PSUM->SBUF output copy.
Matmul operands are bf16 (full-rate PE, fp32 PSUM accumulation); inputs are
rounded to bf16 on the host so they can be DMA'd directly.
"""

import sys

for _p in ("/opt/trn_rl_repo",):
    if _p not in sys.path:
        sys.path.insert(0, _p)

import ml_dtypes
import numpy as np
from contextlib import ExitStack

import concourse.bass as bass
import concourse.bacc as bacc
import concourse.mybir as mybir
from concourse import tile
from concourse.bass_utils import run_bass_kernel_spmd

NCORES = 8
BS = 256
W = BS // NCORES  # windows per core
N = 256           # tokens per window
DIM = 512
NH = 8
HD = 64
SCALE = HD ** -0.5
F32 = mybir.dt.float32
BF16 = mybir.dt.bfloat16
NPBF = ml_dtypes.bfloat16
EXP = mybir.ActivationFunctionType.Exp
COPY = mybir.ActivationFunctionType.Copy


def build(w_count=W):
    nc = bacc.Bacc(None, target_bir_lowering=False)
    xt = nc.declare_dram_parameter("xt", [w_count, DIM, N], BF16, False)
    wqk = nc.declare_dram_parameter("wqk", [DIM, 2 * DIM], BF16, False)
    wv = nc.declare_dram_parameter("wv", [DIM, DIM], BF16, False)
    wp = nc.declare_dram_parameter("wp", [DIM, DIM], BF16, False)
    brep = nc.declare_dram_parameter("brep", [128, DIM], F32, False)
    ebt = nc.declare_dram_parameter("ebt", [128, 2 * NH * N], BF16, False)
    ein = nc.declare_dram_parameter("ein", [128, 4, 128], BF16, False)
    ones_c = nc.declare_dram_parameter("ones_c", [128, 32], BF16, False)
    out = nc.declare_dram_parameter("out", [w_count, N, DIM], F32, True)

    with ExitStack() as ctx:
        tc = ctx.enter_context(tile.TileContext(nc))
        const = ctx.enter_context(tc.tile_pool(name="const", bufs=1))
        p_xt = ctx.enter_context(tc.tile_pool(name="xt", bufs=3))
        p_qk = ctx.enter_context(tc.tile_pool(name="qk", bufs=2))
        p_v = ctx.enter_context(tc.tile_pool(name="v", bufs=2))
        p_p = ctx.enter_context(tc.tile_pool(name="pp", bufs=3))
        p_e = ctx.enter_context(tc.tile_pool(name="te", bufs=4))
        p_bc = ctx.enter_context(tc.tile_pool(name="bc", bufs=3))
        p_av = ctx.enter_context(tc.tile_pool(name="av", bufs=3))
        p_rs = ctx.enter_context(tc.tile_pool(name="rs", bufs=2))
        p_o = ctx.enter_context(tc.tile_pool(name="os", bufs=4))
        ps = ctx.enter_context(tc.tile_pool(name="ps", bufs=5, space="PSUM"))
        psp = ctx.enter_context(tc.tile_pool(name="psp", bufs=2, space="PSUM"))
        ps2 = ctx.enter_context(tc.tile_pool(name="ps2", bufs=1, space="PSUM"))

        wqk_s = const.tile([128, 4, 2 * DIM], BF16)
        nc.sync.dma_start(wqk_s[:], wqk.ap().rearrange("(t p) f -> p t f", p=128))
        wv_s = const.tile([128, 4, DIM], BF16)
        nc.sync.dma_start(wv_s[:], wv.ap().rearrange("(t p) f -> p t f", p=128))
        wp_s = const.tile([128, 4, DIM], BF16)
        nc.sync.dma_start(wp_s[:], wp.ap().rearrange("(t p) f -> p t f", p=128))
        br_s = const.tile([128, DIM], F32)
        nc.sync.dma_start(br_s[:], brep.ap())
        eb_s = const.tile([128, 2 * NH * N], BF16)
        nc.sync.dma_start(eb_s[:], ebt.ap())
        ei_s = const.tile([128, 4, 128], BF16)
        nc.sync.dma_start(ei_s[:], ein.ap())
        oc_s = const.tile([128, 32], BF16)
        nc.sync.dma_start(oc_s[:], ones_c.ap())

        xt_ap = xt.ap()
        out_ap = out.ap()

        # pp column index for (head, ktile): per head-pair the layout is
        # (e_k0, e_k1, o_k0, o_k1); score matmuls are ISSUED interleaved
        # e_k0, o_k0, e_k1, o_k1 so adjacent matmuls hit disjoint PE row
        # groups (and rowsum/avT orderings hit disjoint col groups).
        def ppi(h, kt):
            return (h // 2) * 4 + (h % 2) * 2 + kt

        for wp2 in range(w_count // 2):
            w0 = 2 * wp2
            # load xT (channel-major) for both windows: [128, ct, win, tok]
            xt_s = p_xt.tile([128, 4, 2, N], BF16, tag="xt")
            for wi in range(2):
                nc.sync.dma_start(
                    xt_s[:, :, wi, :],
                    xt_ap[w0 + wi].rearrange("(t p) q -> p t q", p=128),
                )

            # qkT[feat, (win tok)] batched over the window pair (N=512 keeps
            # LDWEIGHTS hidden behind the matmul)
            qk_s = p_qk.tile([128, 8, 2, N], BF16, tag="qk")
            for ft in range(8):
                acc = ps.tile([128, 512], F32, tag="ps")
                for ct in range(4):
                    nc.tensor.matmul(
                        acc[:],
                        wqk_s[:, ct, ft * 128:(ft + 1) * 128],
                        xt_s[:, ct, :, :],
                        start=(ct == 0),
                        stop=(ct == 3),
                    )
                nc.scalar.activation(
                    qk_s[:, ft, :, :].rearrange("p a q -> p (a q)"), acc[:], COPY
                )

            # v[tok, feat] (token-major), per window
            v_s = p_v.tile([128, 2, 2, DIM], BF16, tag="v")
            for wi in range(2):
                for kt in range(2):
                    acc = ps.tile([128, 512], F32, tag="ps")
                    for ct in range(4):
                        nc.tensor.matmul(
                            acc[:],
                            xt_s[:, ct, wi, kt * 128:(kt + 1) * 128],
                            wv_s[:, ct, :],
                            start=(ct == 0),
                            stop=(ct == 3),
                        )
                    nc.vector.tensor_copy(
                        v_s[:, wi, kt, :], acc[:]
                    )

            for wi in range(2):
                w = w0 + wi
                # scores^T: head pairs interleaved into a 2-bank PSUM tile,
                # one batched exp + bias-multiply per pair
                pp_s = p_p.tile([128, 2 * NH * N], BF16, tag="pp")
                for hp in range(4):
                    scpE = psp.tile([128, 2, N], F32, tag="scp")
                    scpO = psp.tile([128, 2, N], F32, tag="scp")
                    scp = [scpE, scpO]
                    for kt in range(2):
                        for s in range(2):
                            h = 2 * hp + s
                            base = s * HD
                            nc.tensor.matmul(
                                scp[s][:, kt, :],
                                qk_s[base:base + HD, 4 + hp, wi,
                                     kt * 128:(kt + 1) * 128],
                                qk_s[base:base + HD, hp, wi, :],
                                start=True,
                                stop=True,
                            )
                    for s in range(2):
                        te = p_e.tile([128, 2 * N], BF16, tag="te")
                        nc.scalar.activation(
                            te[:], scp[s][:].rearrange("p a q -> p (a q)"),
                            EXP, scale=SCALE,
                        )
                        off = (hp * 4 + s * 2) * N
                        nc.vector.tensor_mul(
                            pp_s[:, off:off + 2 * N],
                            te[:],
                            eb_s[:, off:off + 2 * N],
                        )

                # rowsums: ones-block matmuls, kt-outer so consecutive
                # matmuls hit disjoint PE column groups; heads 0-3 ->
                # rsAB[:,0,:], heads 4-7 -> rsAB[:,1,:]
                rsAB = ps2.tile([128, 2, N], F32, tag="rsAB")
                for h in range(NH):
                    pos = 32 * (h % 4)
                    for kt in range(2):
                        nc.tensor.matmul(
                            rsAB[pos:pos + 32, h // 4, :],
                            oc_s[:],
                            pp_s[:, ppi(h, kt) * N:(ppi(h, kt) + 1) * N],
                            start=(kt == 0),
                            stop=(kt == 1),
                            tile_position=(0, pos),
                        )

                # reciprocal (fp32 fast-approx) + one bf16 cast;
                # broadcast to feature rows via indicator matmul
                rc = p_rs.tile([128, 512], F32, tag="rc")
                nc.vector.reciprocal_approx_fast(
                    rc[:], rsAB[:].rearrange("p a q -> p (a q)")
                )
                rcb = p_rs.tile([128, 512], BF16, tag="rcb")
                nc.vector.tensor_copy(rcb[:], rc[:])
                bc_s = p_bc.tile([128, 4 * N], F32, tag="bc")
                for bp2 in range(2):
                    bb = ps.tile([128, 512], F32, tag="ps")
                    for sub in range(2):
                        ftl = 2 * bp2 + sub
                        nc.tensor.matmul(
                            bb[:, sub * N:(sub + 1) * N],
                            ei_s[:, ftl, :],
                            rcb[:, (ftl // 2) * N:(ftl // 2 + 1) * N],
                            start=True,
                            stop=True,
                        )
                    nc.scalar.activation(
                        bc_s[:, bp2 * 512:(bp2 + 1) * 512], bb[:], COPY
                    )

                # avT[f, q]: head pairs in PE column groups, kt-inner order
                # e_k0, o_k0, e_k1, o_k1 for col-group overlap; two feature
                # tiles share a PSUM bank -> one batched normalize-mult
                av_s = p_av.tile([128, 4 * N], BF16, tag="av")
                for ap2 in range(2):
                    aa = ps.tile([128, 512], F32, tag="ps")
                    for sub2 in range(2):
                        ftl = 2 * ap2 + sub2
                        for sub in range(2):
                            h = 2 * ftl + sub
                            for kt in range(2):
                                nc.tensor.matmul(
                                    aa[sub * HD:(sub + 1) * HD,
                                       sub2 * N:(sub2 + 1) * N],
                                    v_s[:, wi, kt, h * HD:(h + 1) * HD],
                                    pp_s[:, ppi(h, kt) * N:(ppi(h, kt) + 1) * N],
                                    start=(kt == 0),
                                    stop=(kt == 1),
                                    tile_position=(0, sub * HD),
                                )
                    nc.vector.tensor_mul(
                        av_s[:, ap2 * 512:(ap2 + 1) * 512],
                        aa[:],
                        bc_s[:, ap2 * 512:(ap2 + 1) * 512],
                    )

                # projection; add (pre-broadcast) b_proj during the
                # PSUM->SBUF copy; output DMA on the gpsimd queue so it
                # never head-of-line-blocks the next window's xT load
                for qt in range(2):
                    oo = ps.tile([128, 512], F32, tag="ps")
                    for ftl in range(4):
                        nc.tensor.matmul(
                            oo[:],
                            av_s[:, ftl * N + qt * 128:ftl * N + qt * 128 + 128],
                            wp_s[:, ftl, :],
                            start=(ftl == 0),
                            stop=(ftl == 3),
                        )
                    o_s = p_o.tile([128, DIM], F32, tag="os")
                    nc.vector.tensor_add(o_s[:], oo[:], br_s[:])
                    nc.gpsimd.dma_start(
                        out_ap[w, qt * 128:(qt + 1) * 128, :], o_s[:]
                    )

    nc.finalize()
    return nc


_NC_CACHE = {}


def _get_nc(w_count):
    if w_count not in _NC_CACHE:
        _NC_CACHE[w_count] = build(w_count)
    return _NC_CACHE[w_count]


def _prep(inputs, w_count):
    x = np.asarray(inputs["x"], dtype=np.float32)
    noise = np.asarray(inputs["noise"], dtype=np.float32)
    ns = np.asarray(inputs["noise_strength"], dtype=np.float32)
    wqkv = np.asarray(inputs["w_qkv"], dtype=np.float32)
    wproj = np.asarray(inputs["w_proj"], dtype=np.float32)
    bproj = np.asarray(inputs["b_proj"], dtype=np.float32)
    bt = np.asarray(inputs["bias_table"], dtype=np.float32)
    ri = np.asarray(inputs["rel_index"])

    xe = x + noise * ns                                     # [BS, N, DIM]
    xt = np.ascontiguousarray(xe.transpose(0, 2, 1).astype(NPBF))
    eb = np.exp(bt[ri])                                     # [q, k, h]
    ebT = eb.transpose(2, 1, 0)                             # [h, k, q]
    ebt = np.zeros((128, 2 * NH, N), np.float32)
    for h in range(NH):
        for kt in range(2):
            i = (h // 2) * 4 + (h % 2) * 2 + kt
            ebt[:, i, :] = ebT[h, kt * 128:(kt + 1) * 128, :]
    ebt = np.ascontiguousarray(ebt.reshape(128, 2 * NH * N).astype(NPBF))
    ein = np.zeros((128, 4, 128), NPBF)
    for ftl in range(4):
        for fl in range(128):
            head = 2 * ftl + (fl // HD)
            ein[32 * (head % 4), ftl, fl] = 1.0

    common = {
        "wqk": np.ascontiguousarray(wqkv[:, : 2 * DIM].astype(NPBF)),
        "wv": np.ascontiguousarray(wqkv[:, 2 * DIM:].astype(NPBF)),
        "wp": np.ascontiguousarray(wproj.astype(NPBF)),
        "brep": np.ascontiguousarray(
            np.broadcast_to(bproj.reshape(1, DIM), (128, DIM)).astype(np.float32)
        ),
        "ebt": ebt,
        "ein": ein,
        "ones_c": np.ones((128, 32), NPBF),
    }
    in_maps = []
    for i in range(NCORES):
        m = dict(common)
        m["xt"] = np.ascontiguousarray(xt[i * w_count:(i + 1) * w_count])
        in_maps.append(m)
    return in_maps


def _run(inputs, w_count=W, trace=False):
    nc = _get_nc(w_count)
    in_maps = _prep(inputs, w_count)
    res = run_bass_kernel_spmd(
        nc, in_maps, core_ids=list(range(NCORES)), trace=trace
    )
    full = np.concatenate([res.results[i]["out"] for i in range(NCORES)], axis=0)
    return full, res


def kernel(**inputs):
    out, _ = _run(inputs, W, trace=False)
    return out


def kernel_profiled(inputs, w_count=W):
    out, res = _run(inputs, w_count, trace=True)
    return out, res


# revision 40
# speedup vs baseline: 12.5832x; 1.1674x over previous
"""Windowed (Swin-style) multi-head attention on 8 TRN2 NeuronCores.

Data-parallel: 256 independent windows -> 32 per core. Per window:
  qkv = x @ w_qkv ; per-head attn = softmax(q k^T * scale + bias) ; out = (attn v) @ w_proj + b_proj

Device-side layout strategy (all matmuls contract over the partition dim):
  - host pre-transposes x to channel-major xT[c, tok] so qT/kT are produced
    feature-major (ready to be score-matmul operands) and v token-major.
  - scores are computed TRANSPOSED, S^T[k, q] (lhsT = kT tile, rhs = qT), so
    softmax normalization runs over the partition axis:
      exp via ScalarE (scale folded in), * exp(bias) via VectorE,
      column-sums via ones-block matmul on TensorE (32 replicated rows at
      32-aligned partitions), reciprocal_approx_fast on VectorE, broadcast
      back to feature rows via an indicator matmul.
  - avT[f, q] = v-contracted matmul accumulated over k tiles; normalized
    avT is exactly the lhsT the projection matmul needs. b_proj is added
    (pre-broadcast on host) during the PSUM->SBUF output copy.
Matmul operands are bf16 (full-rate PE, fp32 PSUM accumulation); inputs are
rounded to bf16 on the host so they can be DMA'd directly.
"""

import sys

for _p in ("/opt/trn_rl_repo",):
    if _p not in sys.path:
        sys.path.insert(0, _p)

import ml_dtypes
import numpy as np
from contextlib import ExitStack

import concourse.bass as bass
import concourse.bacc as bacc
import concourse.mybir as mybir
from concourse import tile
from concourse.bass_utils import run_bass_kernel_spmd

NCORES = 8
BS = 256
W = BS // NCORES  # windows per core
N = 256           # tokens per window
DIM = 512
NH = 8
HD = 64
SCALE = HD ** -0.5
F32 = mybir.dt.float32
BF16 = mybir.dt.bfloat16
NPBF = ml_dtypes.bfloat16
EXP = mybir.ActivationFunctionType.Exp
COPY = mybir.ActivationFunctionType.Copy


def build(w_count=W):
    nc = bacc.Bacc(None, target_bir_lowering=False)
    xt = nc.declare_dram_parameter("xt", [w_count, DIM, N], BF16, False)
    wqk = nc.declare_dram_parameter("wqk", [DIM, 2 * DIM], BF16, False)
    wv = nc.declare_dram_parameter("wv", [DIM, DIM], BF16, False)
    wp = nc.declare_dram_parameter("wp", [DIM, DIM], BF16, False)
    brep = nc.declare_dram_parameter("brep", [128, DIM], F32, False)
    ebt = nc.declare_dram_parameter("ebt", [128, 2 * NH * N], BF16, False)
    ein = nc.declare_dram_parameter("ein", [128, 4, 128], BF16, False)
    ones_c = nc.declare_dram_parameter("ones_c", [128, 32], BF16, False)
    out = nc.declare_dram_parameter("out", [w_count, N, DIM], F32, True)

    with ExitStack() as ctx:
        tc = ctx.enter_context(tile.TileContext(nc))
        const = ctx.enter_context(tc.tile_pool(name="const", bufs=1))
        p_xt = ctx.enter_context(tc.tile_pool(name="xt", bufs=3))
        p_qk = ctx.enter_context(tc.tile_pool(name="qk", bufs=2))
        p_v = ctx.enter_context(tc.tile_pool(name="v", bufs=2))
        p_p = ctx.enter_context(tc.tile_pool(name="pp", bufs=3))
        p_e = ctx.enter_context(tc.tile_pool(name="te", bufs=4))
        p_bc = ctx.enter_context(tc.tile_pool(name="bc", bufs=3))
        p_av = ctx.enter_context(tc.tile_pool(name="av", bufs=3))
        p_rs = ctx.enter_context(tc.tile_pool(name="rs", bufs=2))
        p_o = ctx.enter_context(tc.tile_pool(name="os", bufs=4))
        ps = ctx.enter_context(tc.tile_pool(name="ps", bufs=2, space="PSUM"))
        psp = ctx.enter_context(tc.tile_pool(name="psp", bufs=2, space="PSUM"))
        ps2 = ctx.enter_context(tc.tile_pool(name="ps2", bufs=2, space="PSUM"))

        wqk_s = const.tile([128, 4, 2 * DIM], BF16)
        nc.sync.dma_start(wqk_s[:], wqk.ap().rearrange("(t p) f -> p t f", p=128))
        wv_s = const.tile([128, 4, DIM], BF16)
        nc.sync.dma_start(wv_s[:], wv.ap().rearrange("(t p) f -> p t f", p=128))
        wp_s = const.tile([128, 4, DIM], BF16)
        nc.sync.dma_start(wp_s[:], wp.ap().rearrange("(t p) f -> p t f", p=128))
        br_s = const.tile([128, DIM], F32)
        nc.sync.dma_start(br_s[:], brep.ap())
        eb_s = const.tile([128, 2 * NH * N], BF16)
        nc.sync.dma_start(eb_s[:], ebt.ap())
        ei_s = const.tile([128, 4, 128], BF16)
        nc.sync.dma_start(ei_s[:], ein.ap())
        oc_s = const.tile([128, 32], BF16)
        nc.sync.dma_start(oc_s[:], ones_c.ap())

        xt_ap = xt.ap()
        out_ap = out.ap()

        # pp column index for (head, ktile): per head-pair the layout is
        # (e_k0, e_k1, o_k0, o_k1); score matmuls are ISSUED interleaved
        # e_k0, o_k0, e_k1, o_k1 so adjacent matmuls hit disjoint PE row
        # groups (and rowsum/avT orderings hit disjoint col groups).
        def ppi(h, kt):
            return (h // 2) * 4 + (h % 2) * 2 + kt

        for wp2 in range(w_count // 2):
            w0 = 2 * wp2
            # load xT (channel-major) for both windows: [128, ct, win, tok]
            xt_s = p_xt.tile([128, 4, 2, N], BF16, tag="xt")
            for wl in range(2):
                nc.sync.dma_start(
                    xt_s[:, :, wl, :],
                    xt_ap[w0 + wl].rearrange("(t p) q -> p t q", p=128),
                )

            # qkT[feat, (win tok)] batched over the window pair (N=512 keeps
            # LDWEIGHTS hidden behind the matmul)
            qk_s = p_qk.tile([128, 8, 2, N], BF16, tag="qk")
            for ft in range(8):
                acc = ps.tile([128, 512], F32, tag="ps")
                for ct in range(4):
                    nc.tensor.matmul(
                        acc[:],
                        wqk_s[:, ct, ft * 128:(ft + 1) * 128],
                        xt_s[:, ct, :, :],
                        start=(ct == 0),
                        stop=(ct == 3),
                    )
                nc.scalar.activation(
                    qk_s[:, ft, :, :].rearrange("p a q -> p (a q)"), acc[:], COPY
                )

            # v[tok, feat] (token-major), per window
            v_s = p_v.tile([128, 2, 2, DIM], BF16, tag="v")
            for wi in range(2):
                for kt in range(2):
                    acc = ps.tile([128, 512], F32, tag="ps")
                    for ct in range(4):
                        nc.tensor.matmul(
                            acc[:],
                            xt_s[:, ct, wi, kt * 128:(kt + 1) * 128],
                            wv_s[:, ct, :],
                            start=(ct == 0),
                            stop=(ct == 3),
                        )
                    nc.vector.tensor_copy(
                        v_s[:, wi, kt, :], acc[:]
                    )

            # phases per window, emitted interleaved (S0 S1 R0 A0 P0 R1 A1
            # P1) so each in-order engine always has ready work queued ahead
            # of any dependency-stalled op.
            def phase_scores(wi):
                # scores^T -> exp(scale*s) * exp(bias) -> pp; then rowsums
                pp_s = p_p.tile([128, 2 * NH * N], BF16, tag="pp",
                                name=f"pp_{wi}")
                for hp in range(4):
                    scpE = psp.tile([128, 2, N], F32, tag="scp")
                    scpO = psp.tile([128, 2, N], F32, tag="scp")
                    scp = [scpE, scpO]
                    for kt in range(2):
                        for s in range(2):
                            h = 2 * hp + s
                            base = s * HD
                            nc.tensor.matmul(
                                scp[s][:, kt, :],
                                qk_s[base:base + HD, 4 + hp, wi,
                                     kt * 128:(kt + 1) * 128],
                                qk_s[base:base + HD, hp, wi, :],
                                start=True,
                                stop=True,
                            )
                    for s in range(2):
                        te = p_e.tile([128, 2 * N], BF16, tag="te",
                                      name=f"te_{wi}_{hp}_{s}")
                        nc.scalar.activation(
                            te[:], scp[s][:].rearrange("p a q -> p (a q)"),
                            EXP, scale=SCALE,
                        )
                        off = (hp * 4 + s * 2) * N
                        nc.vector.tensor_mul(
                            pp_s[:, off:off + 2 * N],
                            te[:],
                            eb_s[:, off:off + 2 * N],
                        )
                rsAB = ps2.tile([128, 2, N], F32, tag="rsAB",
                                name=f"rsAB_{wi}")
                for h in range(NH):
                    pos = 32 * (h % 4)
                    for kt in range(2):
                        nc.tensor.matmul(
                            rsAB[pos:pos + 32, h // 4, :],
                            oc_s[:],
                            pp_s[:, ppi(h, kt) * N:(ppi(h, kt) + 1) * N],
                            start=(kt == 0),
                            stop=(kt == 1),
                            tile_position=(0, pos),
                        )
                return pp_s, rsAB

            def phase_recip(wi, rsAB):
                # reciprocal (fp32 fast-approx) + bf16 cast; broadcast to
                # feature rows via indicator matmul
                rc = p_rs.tile([128, 512], F32, tag="rc", name=f"rc_{wi}")
                nc.vector.reciprocal_approx_fast(
                    rc[:], rsAB[:].rearrange("p a q -> p (a q)")
                )
                rcb = p_rs.tile([128, 512], BF16, tag="rcb", name=f"rcb_{wi}")
                nc.vector.tensor_copy(rcb[:], rc[:])
                bc_s = p_bc.tile([128, 4 * N], F32, tag="bc", name=f"bc_{wi}")
                for bp2 in range(2):
                    bb = ps.tile([128, 512], F32, tag="work", name=f"bb_{wi}")
                    for sub in range(2):
                        ftl = 2 * bp2 + sub
                        nc.tensor.matmul(
                            bb[:, sub * N:(sub + 1) * N],
                            ei_s[:, ftl, :],
                            rcb[:, (ftl // 2) * N:(ftl // 2 + 1) * N],
                            start=True,
                            stop=True,
                        )
                    nc.scalar.activation(
                        bc_s[:, bp2 * 512:(bp2 + 1) * 512], bb[:], COPY
                    )
                return bc_s

            def phase_av(wi, pp_s, bc_s):
                # avT[f, q]: head pairs in PE column groups; batched
                # normalize-mult per two feature tiles
                av_s = p_av.tile([128, 4 * N], BF16, tag="av",
                                 name=f"av_{wi}")
                for ap2 in range(2):
                    aa = ps.tile([128, 512], F32, tag="work",
                                 name=f"aa_{wi}_{ap2}")
                    for sub2 in range(2):
                        ftl = 2 * ap2 + sub2
                        for sub in range(2):
                            h = 2 * ftl + sub
                            for kt in range(2):
                                nc.tensor.matmul(
                                    aa[sub * HD:(sub + 1) * HD,
                                       sub2 * N:(sub2 + 1) * N],
                                    v_s[:, wi, kt, h * HD:(h + 1) * HD],
                                    pp_s[:, ppi(h, kt) * N:(ppi(h, kt) + 1) * N],
                                    start=(kt == 0),
                                    stop=(kt == 1),
                                    tile_position=(0, sub * HD),
                                )
                    nc.vector.tensor_mul(
                        av_s[:, ap2 * 512:(ap2 + 1) * 512],
                        aa[:],
                        bc_s[:, ap2 * 512:(ap2 + 1) * 512],
                    )
                return av_s

            def phase_proj(wi, av_s):
                # projection; add b_proj during PSUM->SBUF copy; output DMA
                # on the gpsimd queue (keeps the sync queue free for loads)
                w = w0 + wi
                for qt in range(2):
                    oo = ps.tile([128, 512], F32, tag="work",
                                 name=f"oo_{wi}_{qt}")
                    for ftl in range(4):
                        nc.tensor.matmul(
                            oo[:],
                            av_s[:, ftl * N + qt * 128:ftl * N + qt * 128 + 128],
                            wp_s[:, ftl, :],
                            start=(ftl == 0),
                            stop=(ftl == 3),
                        )
                    o_s = p_o.tile([128, DIM], F32, tag="os",
                                   name=f"os_{wi}_{qt}")
                    nc.vector.tensor_add(o_s[:], oo[:], br_s[:])
                    nc.gpsimd.dma_start(
                        out_ap[w, qt * 128:(qt + 1) * 128, :], o_s[:]
                    )

            pp0, rs0 = phase_scores(0)
            pp1, rs1 = phase_scores(1)
            bc0 = phase_recip(0, rs0)
            av0 = phase_av(0, pp0, bc0)
            phase_proj(0, av0)
            bc1 = phase_recip(1, rs1)
            av1 = phase_av(1, pp1, bc1)
            phase_proj(1, av1)

    nc.finalize()
    return nc


_NC_CACHE = {}


def _get_nc(w_count):
    if w_count not in _NC_CACHE:
        _NC_CACHE[w_count] = build(w_count)
    return _NC_CACHE[w_count]


def _prep(inputs, w_count):
    x = np.asarray(inputs["x"], dtype=np.float32)
    noise = np.asarray(inputs["noise"], dtype=np.float32)
    ns = np.asarray(inputs["noise_strength"], dtype=np.float32)
    wqkv = np.asarray(inputs["w_qkv"], dtype=np.float32)
    wproj = np.asarray(inputs["w_proj"], dtype=np.float32)
    bproj = np.asarray(inputs["b_proj"], dtype=np.float32)
    bt = np.asarray(inputs["bias_table"], dtype=np.float32)
    ri = np.asarray(inputs["rel_index"])

    xe = x + noise * ns                                     # [BS, N, DIM]
    xt = np.ascontiguousarray(xe.transpose(0, 2, 1).astype(NPBF))
    eb = np.exp(bt[ri])                                     # [q, k, h]
    ebT = eb.transpose(2, 1, 0)                             # [h, k, q]
    ebt = np.zeros((128, 2 * NH, N), np.float32)
    for h in range(NH):
        for kt in range(2):
            i = (h // 2) * 4 + (h % 2) * 2 + kt
            ebt[:, i, :] = ebT[h, kt * 128:(kt + 1) * 128, :]
    ebt = np.ascontiguousarray(ebt.reshape(128, 2 * NH * N).astype(NPBF))
    ein = np.zeros((128, 4, 128), NPBF)
    for ftl in range(4):
        for fl in range(128):
            head = 2 * ftl + (fl // HD)
            ein[32 * (head % 4), ftl, fl] = 1.0

    common = {
        "wqk": np.ascontiguousarray(wqkv[:, : 2 * DIM].astype(NPBF)),
        "wv": np.ascontiguousarray(wqkv[:, 2 * DIM:].astype(NPBF)),
        "wp": np.ascontiguousarray(wproj.astype(NPBF)),
        "brep": np.ascontiguousarray(
            np.broadcast_to(bproj.reshape(1, DIM), (128, DIM)).astype(np.float32)
        ),
        "ebt": ebt,
        "ein": ein,
        "ones_c": np.ones((128, 32), NPBF),
    }
    in_maps = []
    for i in range(NCORES):
        m = dict(common)
        m["xt"] = np.ascontiguousarray(xt[i * w_count:(i + 1) * w_count])
        in_maps.append(m)
    return in_maps


def _run(inputs, w_count=W, trace=False):
    nc = _get_nc(w_count)
    in_maps = _prep(inputs, w_count)
    res = run_bass_kernel_spmd(
        nc, in_maps, core_ids=list(range(NCORES)), trace=trace
    )
    full = np.concatenate([res.results[i]["out"] for i in range(NCORES)], axis=0)
    return full, res


def kernel(**inputs):
    out, _ = _run(inputs, W, trace=False)
    return out


def kernel_profiled(inputs, w_count=W):
    out, res = _run(inputs, w_count, trace=True)
    return out, res
